# revision 34
# baseline (speedup 1.0000x reference)
"""AttentionSentGRU Trainium2 kernel (v2: bf16 + unmerged dirs + scan fusion).

Sharding: data-parallel over batch B=128 across 8 cores (BL=16 per core).
Each core runs the full bidirectional GRU + attention pooling + linear
head for its 16 batch elements; no cross-core communication.

Key design vs the fp32 merged baseline:
  - All matmul operands in bf16: fp32 double-pumped matmuls become single
    pass and LDWEIGHTS halves; the recurrent weight reload per step was
    the dominant cost (measured ~213ns per matmul wall on HW).
  - The two GRU directions run as two independent per-step chains
    (separate tiles) so one direction's PE burst overlaps the other
    direction's ACT/DVE gate math.
  - z-gate weights/biases are negated on host, so one sigmoid produces
    (r, zc=1-z) in a single ACT op per direction per step.
  - n-gate math is fused with tensor_tensor_scan pairs:
      scan1 pairs (hn, xn):  e0 state=hn, e1 state = r*hn + xn = narg
      scan2 pairs (Q, h):    e0 state=Q,  e1 state = zc*Q + h = h'
    where Q = nt - h_prev (one DVE op), h' = h + (1-z)(n - h).
  - Hidden state history lives in one interleaved bf16 arena:
      outT col = t_true*64 + dir*32 + b*2 + eo, eo=1 holds h, eo=0 is a
      scratch lane (Q / scan1 even-state) that attention never reads.

Direction convention: 0 = backward (time-reversed on host), 1 = forward.
"""

import numpy as np

B, T, D, H, C = 128, 1024, 256, 128, 10
NCORES = 8
BL = B // NCORES          # batch per core
WT = 16                   # steps per PSUM window

_prog_cache = {}


def _build(Tc):
    import concourse.bass as bass
    import concourse.bacc as bacc
    import concourse.mybir as mybir
    import concourse.tile as tile

    dt = mybir.dt
    AF = mybir.ActivationFunctionType
    ALU = mybir.AluOpType
    AX = mybir.AxisListType

    NW = Tc // WT
    NTOK = BL * Tc
    NJ = NTOK // 512

    nc = bacc.Bacc("TRN2", target_bir_lowering=False, debug=False,
                   num_devices=NCORES)

    f32 = dt.float32
    bf16 = dt.bfloat16

    xt_in = nc.declare_dram_parameter("xt", [2, 2, 128, NW, BL * WT], bf16,
                                      isOutput=False)
    wih_in = nc.declare_dram_parameter("wih", [128, 12 * 128], bf16, isOutput=False)
    whh_in = nc.declare_dram_parameter("whh", [128, 6 * 128], bf16, isOutput=False)
    brz_in = nc.declare_dram_parameter("brz", [4, 128], bf16, isOutput=False)
    srz_in = nc.declare_dram_parameter("srz", [4, 64 * WT], bf16, isOutput=False)
    bna_in = nc.declare_dram_parameter("bna", [4, 128], bf16, isOutput=False)
    sna_in = nc.declare_dram_parameter("sna", [4, 64 * WT], bf16, isOutput=False)
    bihn_in = nc.declare_dram_parameter("bihn", [128, 2], f32, isOutput=False)
    wsent_in = nc.declare_dram_parameter("wsent", [128, 4 * 128], bf16, isOutput=False)
    bsent_in = nc.declare_dram_parameter("bsent", [128, 2], f32, isOutput=False)
    qv_in = nc.declare_dram_parameter("qv", [128, 2], bf16, isOutput=False)
    wlin_in = nc.declare_dram_parameter("wlin", [128, 2 * C], f32, isOutput=False)
    blin_in = nc.declare_dram_parameter("blin", [1, C], f32, isOutput=False)
    out_lg = nc.declare_dram_parameter("logits", [BL, C], f32, isOutput=True)
    import os
    dbg = os.environ.get("GRU_DEBUG") == "1"
    if dbg:
        dbg_out = nc.declare_dram_parameter("dbg_out", [128, 64 * Tc], f32,
                                            isOutput=True)

    with tile.TileContext(nc) as tc:
        with (
            tc.tile_pool(name="cst", bufs=1) as cst,
            tc.tile_pool(name="big", bufs=1) as big,
            tc.tile_pool(name="sbw", bufs=2) as sbw,
            tc.tile_pool(name="stp", bufs=6) as stp,
            tc.tile_pool(name="att", bufs=2) as att,
            tc.tile_pool(name="ps", bufs=2, space="PSUM") as ps,
            tc.tile_pool(name="dramp", bufs=1, space="DRAM") as dramp,
        ):
            sdram = dramp.tile([NTOK], f32)
            sumdram = dramp.tile([BL], f32)
            edram = dramp.tile([NTOK], f32)

            # ---- constants to SBUF ----
            wih = cst.tile([128, 12 * 128], bf16)
            nc.sync.dma_start(wih[:], wih_in[:])
            whh = cst.tile([128, 6 * 128], bf16)
            nc.sync.dma_start(whh[:], whh_in[:])
            brz = cst.tile([4, 128], bf16)
            nc.sync.dma_start(brz[:], brz_in[:])
            srz = cst.tile([4, 64 * WT], bf16)
            nc.sync.dma_start(srz[:], srz_in[:])
            bna = cst.tile([4, 128], bf16)
            nc.sync.dma_start(bna[:], bna_in[:])
            sna = cst.tile([4, 64 * WT], bf16)
            nc.sync.dma_start(sna[:], sna_in[:])
            bihn = cst.tile([128, 2], f32)
            nc.sync.dma_start(bihn[:], bihn_in[:])
            wsent = cst.tile([128, 4 * 128], bf16)
            nc.sync.dma_start(wsent[:], wsent_in[:])
            bsent = cst.tile([128, 2], f32)
            nc.sync.dma_start(bsent[:], bsent_in[:])
            qv = cst.tile([128, 2], bf16)
            nc.sync.dma_start(qv[:], qv_in[:])
            wlin = cst.tile([128, 2 * C], f32)
            nc.sync.dma_start(wlin[:], wlin_in[:])
            blin = cst.tile([1, C], f32)
            nc.sync.dma_start(blin[:], blin_in[:])
            ones1 = cst.tile([1, BL], f32)
            nc.vector.memset(ones1[:], 1.0)
            ones128 = cst.tile([128, 1], f32)
            nc.vector.memset(ones128[:], 1.0)

            def wih_c(dirn, mc, kc):
                i = (dirn * 3 + mc) * 2 + kc
                return wih[:, i * 128:(i + 1) * 128]

            def whh_c(dirn, mc):
                i = dirn * 3 + mc
                return whh[:, i * 128:(i + 1) * 128]

            # outT: interleaved h arena, col = t*64 + dir*32 + b*2 + eo
            outT = big.tile([128, 64 * Tc], bf16)
            opitch = list(outT[:].ap[0])
            otensor = outT[:].tensor

            def blk_off(dirn, t):
                tt = t if dirn == 1 else (Tc - 1 - t)
                return tt * 64 + dirn * 32

            def blk(dirn, t):
                return bass.AP(tensor=otensor, offset=blk_off(dirn, t),
                               ap=[opitch, [1, 32]])

            def blk_h(dirn, t):
                return bass.AP(tensor=otensor, offset=blk_off(dirn, t) + 1,
                               ap=[opitch, [2, BL]])

            def blk_q(dirn, t):
                return bass.AP(tensor=otensor, offset=blk_off(dirn, t),
                               ap=[opitch, [2, BL]])

            # step-0 pseudo-block: evens get Q_0, odds stay 0 (= h_{-1})
            zblk = cst.tile([128, 64], bf16)
            nc.vector.memset(zblk[:], 0.0)
            zt = zblk[:].tensor
            zp0 = list(zblk[:].ap[0])

            def zblk_full(dirn):
                return bass.AP(tensor=zt, offset=dirn * 32, ap=[zp0, [1, 32]])

            def zblk_h(dirn):
                return bass.AP(tensor=zt, offset=dirn * 32 + 1, ap=[zp0, [2, BL]])

            def zblk_q(dirn):
                return bass.AP(tensor=zt, offset=dirn * 32, ap=[zp0, [2, BL]])

            # per-dir (r, zc) tile: col = g*32 + b*2 + eo ; evens stay zero
            ZP = []
            for dirn in (0, 1):
                z = cst.tile([128, 64], f32, name=f"zp{dirn}")
                nc.vector.memset(z[:], 0.0)
                ZP.append(z)

            def zp_odds(dirn):
                z = ZP[dirn]
                return bass.AP(tensor=z[:].tensor, offset=1,
                               ap=[list(z[:].ap[0]), [32, 2], [2, BL]])

            def zp_r(dirn):
                z = ZP[dirn]
                return bass.AP(tensor=z[:].tensor, offset=0,
                               ap=[list(z[:].ap[0]), [1, 32]])

            def zp_zc(dirn):
                z = ZP[dirn]
                return bass.AP(tensor=z[:].tensor, offset=32,
                               ap=[list(z[:].ap[0]), [1, 32]])

            windows = {}

            def win_dma(w):
                """Issue window w's x DMAs two windows ahead of use, so the
                WAR chain on the 3-deep xw buffers never gates the prep."""
                xw = {}
                for dirn in (0, 1):
                    for kc in (0, 1):
                        xt = sbw.tile([128, BL * WT], bf16,
                                      tag=f"xw{dirn}{kc}",
                                      name=f"xw{dirn}{kc}_{w}")
                        nc.sync.dma_start(xt[:], xt_in[:][dirn, kc, :, w, :])
                        xw[(dirn, kc)] = xt
                windows[w] = {"w": w, "xw": xw}

            def win_prep(w, slot):
                """Emit one slice of window w's preparation (slots 1..15),
                spread one-per-step across the preceding window so the PE
                ops slot into chain-wait idle time instead of bursting.
                Layouts: PW rz col = dir*512 + g*256 + tl*16 + b (0=r,1=zneg);
                per-dir n arena col = (tl*16+b)*2 + eo (0=hn, 1=xn).  xn
                projects into PW scratch first (before PW's rz use), then a
                DVE strided add interleaves it + b_ih_n into arena odds."""
                st = windows[w]
                if slot == 1:
                    st["PW"] = ps.tile([128, 64 * WT], f32, tag="pw",
                                       name=f"pw{w}")
                    for dirn in (0, 1):
                        st[f"AR{dirn}"] = ps.tile([128, 32 * WT], f32,
                                                  tag=f"ar{dirn}",
                                                  name=f"ar{dirn}_{w}")
                PW, xw = st["PW"], st["xw"]
                if slot in (1, 2, 3, 4, 5, 6):
                    # per-dir arena fill, 3 slots each:
                    #  a) xn kc0 -> odds, start=True (resets footprint)
                    #  b) bias seed over the full arena, start=False
                    #     (evens := b_hh_n on pending-zero, odds += b_ih_n)
                    #  c) xn kc1 -> odds, start=False (two halves: a single
                    #     [2,256] accumulate trips CoreSim's extent check)
                    dirn, ph = (slot - 1) // 3, (slot - 1) % 3
                    AR = st[f"AR{dirn}"]
                    artens = AR[:].tensor
                    arp0 = list(AR[:].ap[0])
                    if ph == 0:
                        xn_dst = bass.AP(tensor=artens, offset=1,
                                         ap=[arp0, [2, BL * WT]])
                        nc.tensor.matmul(xn_dst, wih_c(dirn, 2, 0),
                                         xw[(dirn, 0)][:],
                                         start=True, stop=False,
                                         skip_group_check=True)
                    elif ph == 1:
                        nc.tensor.matmul(AR[:], bna[:],
                                         sna[:, dirn * 512:(dirn + 1) * 512],
                                         start=False, stop=False,
                                         skip_group_check=True)
                    else:
                        for hh in (0, 1):
                            xn_dst = bass.AP(tensor=artens,
                                             offset=hh * 256 + 1,
                                             ap=[arp0, [2, BL * WT // 2]])
                            nc.tensor.matmul(
                                xn_dst, wih_c(dirn, 2, 1),
                                xw[(dirn, 1)][:, hh * 128:(hh + 1) * 128],
                                start=False, stop=(hh == 1),
                                skip_group_check=True)
                elif slot == 8:
                    for hh in (0, 1):
                        nc.tensor.matmul(PW[:, hh * 512:(hh + 1) * 512],
                                         brz[:],
                                         srz[:, hh * 512:(hh + 1) * 512],
                                         start=True, stop=False,
                                         skip_group_check=True)
                else:
                    idxmap = {7: [], 9: [0], 10: [1], 11: [2], 12: [3],
                              13: [4], 14: [5], 15: [6, 7]}
                    for idx in idxmap[slot]:
                        dirn, g, kc = idx // 4, (idx // 2) % 2, idx % 2
                        off = dirn * 512 + g * 256
                        nc.tensor.matmul(PW[:, off:off + 256],
                                         wih_c(dirn, g, kc),
                                         xw[(dirn, kc)][:],
                                         start=False, stop=False,
                                         skip_group_check=True)

            def half_step(dirn, t):
                w, tl = t // WT, t % WT
                W_ = windows[w]
                PW, AR = W_["PW"], W_[f"AR{dirn}"]
                h_prev = zblk_h(dirn) if t == 0 else blk_h(dirn, t - 1)

                # recurrent matmuls: r, zneg -> PW slices; hn -> arena evens
                for g in (0, 1):
                    off = dirn * 512 + g * 256 + tl * 16
                    nc.tensor.matmul(PW[:, off:off + 16], whh_c(dirn, g),
                                     h_prev, start=False, stop=True,
                                     skip_group_check=True)
                hn_dst = bass.AP(tensor=AR[:].tensor, offset=tl * 32,
                                 ap=[list(AR[:].ap[0]), [2, BL]])
                nc.tensor.matmul(hn_dst, whh_c(dirn, 2), h_prev,
                                 start=False, stop=True, skip_group_check=True)

                # sigma over (r | zneg): writes (r, zc) into ZP odds
                sig_in = bass.AP(tensor=PW[:].tensor,
                                 offset=dirn * 512 + tl * 16,
                                 ap=[list(PW[:].ap[0]), [256, 2], [1, 16]])
                nc.scalar.activation(zp_odds(dirn), sig_in, AF.Sigmoid)

                # scan1: e0 state=hn, e1 state=r*hn+xn -> narg at odds
                narg = stp.tile([128, 32], bf16, tag=f"na{dirn}",
                                name=f"na{dirn}_{t}")
                ar_sl = bass.AP(tensor=AR[:].tensor, offset=tl * 32,
                                ap=[list(AR[:].ap[0]), [1, 32]])
                nc.vector.tensor_tensor_scan(narg[:], zp_r(dirn), ar_sl, 0.0,
                                             op0=ALU.mult, op1=ALU.add)

                # tanh on odds -> nt
                nt = stp.tile([128, 16], bf16, tag=f"nt{dirn}",
                              name=f"nt{dirn}_{t}")
                narg_odds = bass.AP(tensor=narg[:].tensor, offset=1,
                                    ap=[list(narg[:].ap[0]), [2, 16]])
                nc.scalar.activation(nt[:], narg_odds, AF.Tanh)

                # Q = nt - h_prev -> evens of previous block
                q_dst = zblk_q(dirn) if t == 0 else blk_q(dirn, t - 1)
                nc.vector.tensor_tensor(q_dst, nt[:], h_prev, ALU.subtract)

                # scan2: e0 state=Q, e1 state=zc*Q+h -> new block
                d1 = zblk_full(dirn) if t == 0 else blk(dirn, t - 1)
                nc.vector.tensor_tensor_scan(blk(dirn, t), zp_zc(dirn), d1,
                                             0.0, op0=ALU.mult, op1=ALU.add)

            # ---------------- main loop ----------------
            PREF = 4
            win_dma(0)
            for slot in range(1, WT):
                win_prep(0, slot)
            for s in range(Tc + 1):
                if s % WT == PREF and (s // WT) + 1 < NW:
                    wn = s // WT + 1
                    win_dma(wn)
                    for slot in range(1, WT):
                        win_prep(wn, slot)
                if s < Tc:
                    half_step(1, s)
                if s >= 1:
                    half_step(0, s - 1)

            # ---------------- attention epilogue ----------------
            # u = W_sent^T h; squish = tanh(u + bias); s = q . squish
            def h_span(dirn, j):
                # tokens (t, b) for t in [j*32, (j+1)*32): N = 512
                return bass.AP(tensor=otensor,
                               offset=j * 32 * 64 + dirn * 32 + 1,
                               ap=[opitch, [64, 32], [2, BL]])

            for j in range(NJ):
                PSs = ps.tile([1, 512], f32, tag="ar0", name=f"pss{j}")
                for mc in (0, 1):
                    PU = ps.tile([128, 512], f32, tag="pw", name=f"pu{j}_{mc}")
                    for kc in (0, 1):
                        rhs = h_span(1 - kc, j)
                        nc.tensor.matmul(PU[:], wsent[:, (kc * 2 + mc) * 128:
                                                      (kc * 2 + mc + 1) * 128],
                                         rhs, start=(kc == 0), stop=(kc == 1))
                    tu = att.tile([128, 512], bf16, tag="tu", name=f"tu{j}_{mc}")
                    nc.scalar.activation(tu[:], PU[:], AF.Tanh,
                                         bias=bsent[:, mc:mc + 1])
                    nc.tensor.matmul(PSs[:], qv[:, mc:mc + 1], tu[:],
                                     start=(mc == 0), stop=(mc == 1))
                sbounce = att.tile([1, 512], f32, tag="sbounce", name=f"sb{j}")
                nc.vector.tensor_copy(sbounce[:], PSs[:])
                nc.sync.dma_start(sdram[j * 512:(j + 1) * 512], sbounce[:])

            # softmax denominator per batch element (token idx = t*16 + b)
            PT = min(128, NTOK // BL)
            CT = NTOK // (PT * BL)
            s2d = cst.tile([PT, CT * BL], f32, name="s2d")
            nc.sync.dma_start(s2d[:], sdram[:].rearrange("(p c) -> p c", p=PT))
            es = cst.tile([PT, CT * BL], f32, name="es")
            nc.scalar.activation(es[:], s2d[:], AF.Exp)
            part = cst.tile([PT, BL], f32, name="part")
            nc.vector.tensor_reduce(
                part[:],
                es[:].rearrange("p (ct b) -> p b ct", b=BL),
                axis=AX.X, op=ALU.add)
            onesPT = cst.tile([PT, 1], f32, name="onesPT")
            nc.vector.memset(onesPT[:], 1.0)
            PSb = ps.tile([1, BL], f32, tag="ar1", name="psb")
            nc.tensor.matmul(PSb[:], onesPT[:], part[:], start=True, stop=True)
            bsum = cst.tile([1, BL], f32, name="bsum")
            nc.vector.tensor_copy(bsum[:], PSb[:])
            nc.sync.dma_start(sumdram[:], bsum[:])
            brow = cst.tile([1, BL], f32, name="brow")
            nc.sync.dma_start(brow[:], sumdram[:].rearrange("(a b) -> a b", a=1))
            sumb = cst.tile([128, BL], f32, name="sumb")
            nc.gpsimd.partition_broadcast(sumb[:], brow[:])
            rinv = cst.tile([128, BL], f32, name="rinv")
            nc.vector.reciprocal(rinv[:], sumb[:])
            nc.sync.dma_start(edram[:], es[:])

            # pooling: pooled[kc][feat, b] = sum_t h[feat,(t,b)] * e[(t,b)]
            pooled_parts = [cst.tile([128, BL * NJ], f32, name=f"pp{kc}")
                            for kc in (0, 1)]
            for j in range(NJ):
                eb = att.tile([128, 512], f32, tag="eb", name=f"eb{j}")
                nc.sync.dma_start(
                    eb[:],
                    edram[j * 512:(j + 1) * 512].partition_broadcast(128))
                for kc in (0, 1):
                    tmul = att.tile([128, 512], f32, tag="tmul",
                                    name=f"tm{j}_{kc}")
                    nc.vector.tensor_tensor(
                        tmul[:].rearrange("p (t b) -> p t b", b=BL),
                        h_span(1 - kc, j),
                        eb[:].rearrange("p (t b) -> p t b", b=BL), ALU.mult)
                    nc.vector.tensor_reduce(
                        pooled_parts[kc][:, j * BL:(j + 1) * BL],
                        tmul[:].rearrange("p (t b) -> p b t", b=BL),
                        axis=AX.X, op=ALU.add)
            pooledn = [cst.tile([128, BL], f32, name=f"pln{kc}")
                       for kc in (0, 1)]
            for kc in (0, 1):
                pl = cst.tile([128, BL], f32, name=f"pl{kc}")
                nc.vector.tensor_reduce(
                    pl[:],
                    pooled_parts[kc][:].rearrange("p (j b) -> p b j", b=BL),
                    axis=AX.X, op=ALU.add)
                nc.vector.tensor_tensor(pooledn[kc][:], pl[:], rinv[:],
                                        ALU.mult)
            PL = ps.tile([BL, C], f32, tag="ar1", name="pl_ps")
            nc.tensor.matmul(PL[:], ones1[:], blin[:], start=True, stop=False,
                             skip_group_check=True)
            # pooledn[kc]: kc=0 is the forward feature half (lin_w cols 0:128)
            nc.tensor.matmul(PL[:], pooledn[0][:], wlin[:, 0:C],
                             start=False, stop=False, skip_group_check=True)
            nc.tensor.matmul(PL[:], pooledn[1][:], wlin[:, C:2 * C],
                             start=False, stop=True, skip_group_check=True)
            lg = cst.tile([BL, C], f32, name="lg")
            nc.vector.tensor_copy(lg[:], PL[:])
            nc.sync.dma_start(out_lg[:], lg[:])
            if dbg:
                ocp = cst.tile([128, 64 * Tc], f32, name="ocp")
                nc.vector.tensor_copy(ocp[:], outT[:])
                nc.sync.dma_start(dbg_out[:], ocp[:])

    nc.compile()
    return nc


def _pack_inputs(inputs, Tc):
    """Build the 8 per-core input maps. Direction 0 = backward, 1 = forward."""
    import ml_dtypes
    bf = ml_dtypes.bfloat16
    NW = Tc // WT
    x = inputs["word_attn_vectors"][:, :Tc, :]

    w_ih = {0: inputs["w_ih_b"], 1: inputs["w_ih_f"]}
    w_hh = {0: inputs["w_hh_b"], 1: inputs["w_hh_f"]}
    b_ih = {0: inputs["b_ih_b"], 1: inputs["b_ih_f"]}
    b_hh = {0: inputs["b_hh_b"], 1: inputs["b_hh_f"]}

    # gate sign: r,n as-is; z negated (mc order: 0=r, 1=zneg, 2=n)
    gsign = {0: 1.0, 1: -1.0, 2: 1.0}
    gsrc = {0: 0, 1: 1, 2: 2}           # source gate row block in 3H

    wih = np.empty((128, 12 * 128), np.float32)
    for dirn in (0, 1):
        wt = np.ascontiguousarray(w_ih[dirn].T)    # [D, 3H]
        for mc in range(3):
            sg = gsrc[mc]
            for kc in range(2):
                i = (dirn * 3 + mc) * 2 + kc
                wih[:, i * 128:(i + 1) * 128] = gsign[mc] * \
                    wt[kc * 128:(kc + 1) * 128, sg * 128:(sg + 1) * 128]
    whh = np.empty((128, 6 * 128), np.float32)
    for dirn in (0, 1):
        wt = np.ascontiguousarray(w_hh[dirn].T)    # [H, 3H]
        for mc in range(3):
            sg = gsrc[mc]
            i = dirn * 3 + mc
            whh[:, i * 128:(i + 1) * 128] = \
                gsign[mc] * wt[:, sg * 128:(sg + 1) * 128]

    # rz bias seed: class = dir*2 + g, region cols [dir*512 + g*256, +256)
    brz = np.empty((4, 128), np.float32)
    for dirn in (0, 1):
        sbias = (b_ih[dirn] + b_hh[dirn]).astype(np.float32)
        brz[dirn * 2 + 0] = sbias[0:128]
        brz[dirn * 2 + 1] = -sbias[128:256]
    srz = np.zeros((4, 64 * WT), np.float32)
    for dirn in (0, 1):
        for g in (0, 1):
            k = dirn * 2 + g
            srz[k, dirn * 512 + g * 256: dirn * 512 + (g + 1) * 256] = 1.0

    # n arena seed (per-dir arenas): col = (tl*16+b)*2 + eo
    # one seed matmul per arena adds b_hh_n at evens and b_ih_n at odds
    bna = np.zeros((4, 128), np.float32)
    sna = np.zeros((4, 64 * WT), np.float32)
    for dirn in (0, 1):
        for eo in (0, 1):
            k = dirn * 2 + eo
            bna[k] = (b_hh[dirn] if eo == 0 else b_ih[dirn])[256:384]
            sna[k, dirn * 512 + eo: (dirn + 1) * 512: 2] = 1.0
    bihn = np.stack([b_ih[0][256:384], b_ih[1][256:384]],
                    axis=1).astype(np.float32)

    w_sent = inputs["weight_w_sent"]
    bias_sent = inputs["bias_sent"][:, 0]
    qvec = inputs["query_vec_sent"][:, 0]
    wsent = np.empty((128, 4 * 128), np.float32)
    for kc in range(2):
        for mc in range(2):
            wsent[:, (kc * 2 + mc) * 128:(kc * 2 + mc + 1) * 128] = \
                w_sent[kc * 128:(kc + 1) * 128, mc * 128:(mc + 1) * 128]
    bsent = np.stack([bias_sent[0:128], bias_sent[128:256]],
                     axis=1).astype(np.float32)
    qvp = np.stack([qvec[0:128], qvec[128:256]], axis=1).astype(np.float32)

    lin_w = inputs["lin_w"]
    wlin = np.concatenate([lin_w.T[0:128], lin_w.T[128:256]], axis=1)
    wlin = np.ascontiguousarray(wlin, dtype=np.float32)
    blin = inputs["lin_b"].reshape(1, C).astype(np.float32)

    common = dict(wih=wih.astype(bf), whh=whh.astype(bf),
                  brz=brz.astype(bf), srz=srz.astype(bf),
                  bna=bna.astype(bf), sna=sna.astype(bf), bihn=bihn,
                  wsent=wsent.astype(bf), bsent=bsent,
                  qv=qvp.astype(bf), wlin=wlin, blin=blin)

    in_maps = []
    for cc in range(NCORES):
        xc = x[cc * BL:(cc + 1) * BL]               # [BL, Tc, D]
        xdirs = {1: np.ascontiguousarray(xc.transpose(2, 0, 1)),
                 0: np.ascontiguousarray(xc[:, ::-1, :].transpose(2, 0, 1))}
        xt = np.empty((2, 2, 128, NW, BL * WT), np.float32)
        for dirn in (0, 1):
            v = xdirs[dirn].reshape(2, 128, BL, NW, WT)
            xt[dirn] = np.ascontiguousarray(
                v.transpose(0, 1, 3, 4, 2)).reshape(2, 128, NW, BL * WT)
        m = dict(common)
        m["xt"] = xt.astype(bf)
        in_maps.append(m)
    return in_maps


def kernel(**inputs):
    from concourse.bass_utils import run_bass_kernel_spmd

    inputs = {k: np.asarray(v) for k, v in inputs.items()}
    Tc = inputs["word_attn_vectors"].shape[1]
    nc = _prog_cache.get(Tc)
    if nc is None:
        nc = _build(Tc)
        _prog_cache[Tc] = nc
    in_maps = _pack_inputs(inputs, Tc)
    res = run_bass_kernel_spmd(nc, in_maps, core_ids=list(range(NCORES)))
    return np.concatenate([res.results[i]["logits"] for i in range(NCORES)],
                          axis=0).astype(np.float32)
